# revision 43
# baseline (speedup 1.0000x reference)
"""MEX (log-sum-exp) 3x3 pooling kernel for Trainium2, 8-core SPMD.

Math: out[b,m,i,j] = log( (1/n) * sum_{c,dh,dw} exp(x[b,c,i+dh,j+dw] + off[m,c,dh,dw]) )
with n = C*3*3 = 576, eps = 1.

Identity used: the reference's per-pixel max-stabilization cancels exactly:
  out = m_x + m_b + log(S) - log(n)  ==  log( sum_k exp(x_k + b_k) ) - log(n)
Values are benign in fp32/fp16 (x ~ N(0,1) -> exp(x) in [5e-3, 150]; off =
log_softmax values in [-13, -2.5]; weights are pre-scaled by 2^10 so exp fits
fp16 normal range; the scale cancels in the final log).

Per-core plan (core i handles batch images 2i, 2i+1):
  - SBUF E tile (128, 16384+pad) fp16: partition p = img*64 + c, free =
    h*128 + w, E = exp(x) computed by ACT as the x DMA streams in.
  - Weights: off permuted host-side to wp[c, (dh,dw,m)] (64,144); device
    scatters into log-domain LT layout [128, (tap, img, m)] with -80 in the
    cross-image blocks, then exp -> fp16 lhsT per tap (128 x 32).
  - Each 16-row superchunk maps its four consecutive 4-row blocks onto the
    PE's four 32-wide column strips (tile_position=(0,32q)); one PSUM bank
    holds P[32q+im, rl*128+w] = MEX sum for output row 16sc+4q+rl.  The 9
    taps (dh,dw) accumulate via 9 matmuls per strip whose rhs is the flat
    contiguous slice E[:, (16sc+4q+dh)*128+dw : +512].  The 4 strips stream
    concurrently, so the PE does ~9*512 cycles per superchunk, not 36*512.
    A 512-col zero-weight start=True matmul clears each bank's has_written
    bits once (interleaved start groups corrupt each other on TRN2).
  - Post-processing is a single ACT log per superchunk (PSUM -> SBUF fp16,
    free dim 512) and one contiguous output DMA; host un-permutes + crops.
    No vector-engine work at all.  x streams in as fp16 (host-cast).
  - The activation-table dict is filtered so walrus picks the
    natural_log_exp_and_others set (covers Exp AND Ln -> one table load).
"""

import math as _math

import numpy as np

EPS = 1.0
B, C, H, W = 16, 64, 128, 128
M = 16
BH = BW = 3
HO, WO = H - BH + 1, W - BW + 1  # 126, 126
N_TAPS = C * BH * BW  # 576
NCORES = 8
BPC = B // NCORES  # 2 images per core
HWP = H * W  # 16384 pixels per image plane
PAD = 512
SC_ROWS = 16
NSC = H // SC_ROWS  # 8 superchunks of 16 output rows
W_ALPHA_LOG = 10.0 * _math.log(2.0)  # fp16 weight pre-scale (cancels in log)

MM_DTYPE = "f16"  # for test.py's printout

_BUILT = {}


def _build(key="default"):
    """Build (and cache) the Bass/Tile program shared by all 8 cores."""
    if key in _BUILT:
        return _BUILT[key]

    import concourse.bass as bass
    import concourse.bacc as bacc
    import concourse.tile as tile
    from concourse import mybir

    import os as _os

    # Prefer the activation-function set that holds BOTH Exp and Ln so the
    # table-load pass does not thrash between per-function sets.
    _orig_tables = bacc.get_activation_tables
    if _os.environ.get("MEX_NO_TABLE_FIX"):
        pass
    elif getattr(bacc.get_activation_tables, "__name__", "") != "_mex_pref_tables":
        # The set list index IS the runtime table id, so keep dict order and
        # ids intact; just hide Exp/Ln from every other set so the load pass
        # must pick the combined set (one ACT_TABLE_LOAD instead of six).
        def _mex_pref_tables(arch, _orig=_orig_tables):
            from concourse import mybir as _mb

            _AF = _mb.ActivationFunctionType
            t = dict(_orig(arch))
            name = "natural_log_exp_and_others"
            if name in t:
                t = {
                    k: (v if k == name else (set(v) - {_AF.Exp, _AF.Ln}))
                    for k, v in t.items()
                }
            return t

        _mex_pref_tables.__name__ = "_mex_pref_tables"
        bacc.get_activation_tables = _mex_pref_tables

    f32 = mybir.dt.float32
    f16 = mybir.dt.float16
    AF = mybir.ActivationFunctionType

    nc = bacc.Bacc("TRN2", target_bir_lowering=False, debug=False)

    xd = nc.dram_tensor("x", [128, HWP], f16, kind="ExternalInput")
    wpd = nc.dram_tensor("wp", [128, 288], f16, kind="ExternalInput")
    # Output keeps the on-chip [128, sc*512] layout (partition p = 32*q + im
    # holds rows 16*sc+4*q..+3 of image-pair channel im).  One contiguous
    # 128-partition DMA per superchunk; the host un-permutes and crops.
    outd = nc.dram_tensor("out", [128, NSC * 512], f16, kind="ExternalOutput")

    with tile.TileContext(nc) as tc:
        with (
            tc.tile_pool(name="singles", bufs=1) as singles,
            tc.tile_pool(name="xin", bufs=4) as xin,
            tc.tile_pool(name="psum", bufs=7, space="PSUM") as psum,
            tc.tile_pool(name="psumw", bufs=1, space="PSUM") as psumw,
            tc.tile_pool(name="post", bufs=4) as post,
        ):
            # ---- first x chunk DMA before anything else on the Sync queue,
            # weights wp right behind it ----
            E = singles.tile([128, HWP + PAD], f16)
            # chunk boundaries match superchunk needs: sc needs E up to
            # pixel 2048*sc + 2304.  Small leading chunks so the first
            # matmuls (and the HAM clock ramp) start as early as possible.
            xchunks = [2304] + [2048] * 6 + [1536, 256]
            xtiles = []
            xoff = 0
            for npx in xchunks:
                Xk = xin.tile([128, npx], f16, tag="Xk")
                nc.sync.dma_start(
                    out=Xk[:, :], in_=bass.AP(xd, xoff, [[HWP, 128], [1, npx]])
                )
                xtiles.append((Xk, xoff, npx))
                xoff += npx
                if xoff == 2304:
                    LT = singles.tile([128, 288], f16)
                    nc.sync.dma_start(
                        out=LT[:, :], in_=bass.AP(wpd, 0, [[288, 128], [1, 288]])
                    )

            # Zero weights for the 512-column bank-clearing matmul: a single
            # start=True matmul clears the whole 2KB psum bank's has_written
            # bits; the 36 tap matmuls then run start=False (per-element
            # has_written gives overwrite-then-accumulate).  Interleaved
            # start groups in one bank corrupt each other on TRN2.
            ZW = singles.tile([128, 128], f16)
            nc.vector.memset(ZW[:, :], 0.0)
            Zr = singles.tile([128, 512], f16)
            nc.vector.memset(Zr[:, :], 0.0)
            # HAM warm-up: dummy matmuls keep the PE busy from t~7us so the
            # clock gate reaches 2.4 GHz before the first real taps.
            WP_ = psumw.tile([128, 512], f32, tag="warm")
            for wi in range(10):
                nc.tensor.matmul(
                    WP_[0:128, :], ZW[:, 0:128], Zr[:, :],
                    start=(wi == 0), stop=(wi == 9), skip_group_check=True,
                )

            # ---- E = exp(x), streamed.  All exps precede all LNs on the
            # ACT queue (LNs head-of-line block later exps otherwise). ----
            def _emit_exp(idx):
                Xk, off, npx = xtiles[idx]
                nc.scalar.activation(
                    out=E[:, off : off + npx], in_=Xk[:, :], func=AF.Exp, scale=EPS
                )

            _emit_exp(0)

            # pad rows get exp(0)=1 via a DVE memset (keeps ACT free);
            # only superchunk 7 reads them.
            nc.vector.memset(E[:, HWP:], 1.0)

            _emit_exp(1)

            # ---- main loop.  LN(sc) is emitted ~5 exp chunks after sc's
            # matmuls so the static schedule slots each LN into the exp
            # stream once it is ready (no head-of-line blocking), instead of
            # serializing all LNs after the last exp. ----
            ln_scale = 1.0 / (float(N_TAPS) * _math.exp(W_ALPHA_LOG))

            def _emit_ln(sc, P):
                L = post.tile([128, 512], f16, tag="L")
                nc.scalar.activation(
                    out=L[:, :], in_=P[:, :], func=AF.Ln, scale=ln_scale
                )
                dst = bass.AP(outd, sc * 512, [[NSC * 512, 128], [1, 512]])
                nc.gpsimd.dma_start(out=dst, in_=L[:, :])

            ptiles = {}
            for sc in range(NSC):
                if sc + 2 < len(xtiles):
                    _emit_exp(sc + 2)
                P = psum.tile([128, 512], f32, tag="P")
                nc.tensor.matmul(
                    P[0:128, :],
                    ZW[:, 0:128],
                    Zr[:, :],
                    start=True,
                    stop=False,
                    skip_group_check=True,
                )
                for t in range(9):
                    dh, dw = divmod(t, 3)
                    lhsT = LT[:, 32 * t : 32 * (t + 1)]
                    for q in range(4):
                        base = (SC_ROWS * sc + 4 * q + dh) * W + dw
                        nc.tensor.matmul(
                            P[32 * q : 32 * (q + 1), :],
                            lhsT,
                            E[:, base : base + 512],
                            start=False,
                            stop=(t == 8 and q == 3),
                            tile_position=(0, 32 * q),
                            skip_group_check=True,
                        )
                ptiles[sc] = P
                if sc >= 1:
                    _emit_ln(sc - 1, ptiles.pop(sc - 1))
            _emit_ln(NSC - 1, ptiles.pop(NSC - 1))

    nc.compile()
    _BUILT[key] = nc
    return nc


def _prep_inputs(x, offsets):
    x = np.asarray(x, dtype=np.float32)
    off = np.asarray(offsets, dtype=np.float32).reshape(M, C, BH, BW)
    # Host-side exp'd lhsT [p, (tap, img, m)]: wpe[c, t, m] = exp(off + alpha),
    # block-diagonal over img (zeros keep the two images independent).
    wpe = np.exp(
        np.transpose(off, (1, 2, 3, 0)).reshape(64, 9, 16) + W_ALPHA_LOG
    ).astype(np.float16)
    wp = np.zeros((128, 9, 2, 16), dtype=np.float16)
    wp[0:64, :, 0, :] = wpe
    wp[64:128, :, 1, :] = wpe
    wp = np.ascontiguousarray(wp.reshape(128, 288))
    in_maps = [
        {
            "x": np.ascontiguousarray(
                x[BPC * i : BPC * (i + 1)].astype(np.float16)
            ).reshape(128, HWP),
            "wp": wp,
        }
        for i in range(NCORES)
    ]
    return in_maps


def _decode_out(raw):
    """[128, NSC*512] device layout -> (BPC, M, HO, WO) fp32."""
    arr = np.asarray(raw).astype(np.float32).reshape(4, 32, NSC, 4, W)
    # [q, im, sc, rl, w] -> [im, sc, q, rl, w];  row = 16*sc + 4*q + rl
    arr = arr.transpose(1, 2, 0, 3, 4).reshape(32, H, W)
    return arr[:, :HO, :WO].reshape(BPC, M, HO, WO)


def kernel(x, offsets):
    from concourse.bass_utils import run_bass_kernel_spmd

    nc = _build()
    in_maps = _prep_inputs(x, offsets)
    res = run_bass_kernel_spmd(nc, in_maps, core_ids=list(range(NCORES)))
    out = np.empty((B, M, HO, WO), dtype=np.float32)
    for i in range(NCORES):
        out[BPC * i : BPC * (i + 1)] = _decode_out(res.results[i]["out"])
    return out
